# revision 14
# baseline (speedup 1.0000x reference)
"""Trainium2 Bass kernel for CTC batch loss (keras ctc_batch_cost semantics).

Problem: y_true [1024, 32] int labels (blank=95 excluded), y_pred [1024, 256, 96]
softmax-like probs. loss[b] = -logaddexp(alphaT[-1], alphaT[-2]) of the standard
CTC forward DP over logp = log_softmax(log(y_pred + 1e-7)).

Strategy (8 cores, pure data parallel, 128 examples/core, one example per
partition):

  log_softmax(log(p+eps)) = log(q) - log(sum_c q) with q = p + eps, so
      loss = sum_t ln D[t] - ln(aT[S-1] + aT[S-2]),   D[t] = sum_c q[t, c]
  and the DP runs in LINEAR space on q (fp32 range suffices for T=256: the
  trajectories stay within ~1e-30..1e11 on this data distribution).

  The forward DP is reordered label-major: with f_l(t) = alpha(t, 2l+1) and
  g_l(t) = alpha(t, 2l), the recurrences
      g_l(t) = qb(t) * (g_l(t-1) + f_{l-1}(t-1))
      f_l(t) = ql_l(t) * (f_l(t-1) + g_l(t-1) + m_l * f_{l-1}(t-1))
  are per-(example, l) affine scans over t. Each maps onto a single DVE
  tensor_tensor_scan (state = (data0 + state) * data1) of length T=256, so the
  serial chain is 33 * 3 = ~100 wide DVE ops instead of T * 4 short ones.
  The l=0 init is folded in by driving with h_0 = delta(t=0), m_0 = 1.

  Host-side packing writes, per (example, t), a 128-wide row
      [q at labels 0..31 | q at blank | q at classes 0..94]   (bf16)
  so every example's label-l trajectory sits at a fixed column l (no on-device
  gather), and the last 96 columns sum to the exact softmax denominator.
  Loads are chunked t-major and striped across both HW DMA queues (SP + ACT;
  the tiny mask load rides the gpsimd software queue so it does not delay
  chunk 0). Per chunk, the ACT engine upconverts the 33 trajectory columns to
  a contiguous fp32 [PB, 33*256] tile while DVE folds the denominators with a
  bf16 pairwise tree + reduce-24. Scans are windowed to the reachable band
  t in [l-1, T-L+l], and the final Ln is evaluated at 2^56 * TOT (constant
  subtracted in the loss) to stay inside the ACT Ln table's accurate range.

The kernel is self-contained: shapes/sharding hardcoded; inputs are the FULL
arrays as produced by setup_inputs().
"""
import os
import sys
import numpy as np
from contextlib import ExitStack

for _p in ("/opt/trn_rl_repo", "/root/.axon_site/_ro/trn_rl_repo"):
    if os.path.isdir(_p) and _p not in sys.path:
        sys.path.insert(0, _p)

import concourse.bass as bass
import concourse.bacc as bacc
import concourse.tile as tile
from concourse import mybir
from concourse.bass_utils import run_bass_kernel_spmd
from ml_dtypes import bfloat16

B, T, C, L = 1024, 256, 96, 32
NCORES = 8
PB = B // NCORES         # 128 examples per core = one per partition
EPS = np.float32(1e-7)
BLANK = C - 1
W = 128                  # packed row width per t: 32 labels | blank | 95 others
NL = L + 1               # 33 trajectories (labels + blank)
CN = 8                   # legacy full-chunk count (t-major)
CT = T // CN             # 32 time steps per full chunk
CW = CT * W              # elems per full chunk per partition
# non-uniform chunk table (t0, nt): two half chunks first so the vector
# pipeline starts ~3us earlier, then seven full chunks
CHUNKS = [(0, 16), (16, 16)] + [(32 * k, 32) for k in range(1, 8)]

F32 = mybir.dt.float32
BF16 = mybir.dt.bfloat16
ALU = mybir.AluOpType
AF = mybir.ActivationFunctionType


def _pack_core_inputs(yp, yt):
    """yp [128, 256, 96] f32, yt [128, 32] int -> (d3 [CN, PB, CW] bf16,
    m [PB, L] f32)."""
    q = yp.astype(np.float32) + EPS
    d3 = np.empty((PB, T, W), np.float32)
    d3[:, :, 0:L] = np.take_along_axis(q, yt[:, None, :].astype(np.int64), axis=2)
    d3[:, :, L] = q[:, :, BLANK]
    d3[:, :, L + 1:W] = q[:, :, 0:BLANK]
    d3 = d3.reshape(PB, T * W)
    d3a = np.ascontiguousarray(
        d3[:, :CW].reshape(PB, 2, 16 * W).transpose(1, 0, 2)).astype(bfloat16)
    d3b = np.ascontiguousarray(
        d3[:, CW:].reshape(PB, 7, CW).transpose(1, 0, 2)).astype(bfloat16)
    m = np.ones((PB, L), np.float32)
    m[:, 1:] = (yt[:, 1:] != yt[:, :-1]).astype(np.float32)
    return d3a, d3b, m


def build_program():
    nc = bacc.Bacc("TRN2", target_bir_lowering=False, debug=False)
    d3a_d = nc.dram_tensor("d3a", [2, PB, 16 * W], BF16,
                           kind="ExternalInput").ap()
    d3b_d = nc.dram_tensor("d3b", [7, PB, CW], BF16,
                           kind="ExternalInput").ap()
    m_d = nc.dram_tensor("m", [PB, L], F32, kind="ExternalInput").ap()
    loss_d = nc.dram_tensor("loss", [PB, 1], F32, kind="ExternalOutput").ap()

    with ExitStack() as ctx, tile.TileContext(nc) as tc:
        def sb(name, shape, dt=F32):
            return nc.alloc_sbuf_tensor(name, list(shape), dt).ap()

        D3 = sb("D3", [PB, T * W], BF16)
        QL = sb("QL", [PB, NL * T])      # fp32 trajectories, l-major
        MM = sb("MM", [PB, L])
        FD = sb("FD", [PB, T])           # delta drive: h_0
        F0 = sb("F0", [PB, T + 1])       # f ping-pong, col 0 = zero pad
        F1 = sb("F1", [PB, T + 1])
        G = sb("G", [PB, T + 1])
        U = sb("U", [PB, T])
        DT1 = sb("DT1", [PB, CT * 48], BF16)
        DT2 = sb("DT2", [PB, CT * 24], BF16)
        DG = sb("DG", [PB, T])
        LDG = sb("LDG", [PB, T])
        SLD = sb("SLD", [PB, 1])
        TOT = sb("TOT", [PB, 1])
        LNT = sb("LNT", [PB, 1])
        LOSS = sb("LOSS", [PB, 1])
        FF = [F0, F1]

        # --- init (gpsimd: keep the vector queue clear) ---
        nc.gpsimd.memset(FD[:], 0.0)
        nc.gpsimd.memset(FD[:, 0:1], 1.0)
        nc.gpsimd.memset(G[:, 0:1], 0.0)
        nc.gpsimd.memset(F0[:, 0:1], 0.0)
        nc.gpsimd.memset(F1[:, 0:1], 0.0)

        # --- loads: stripe the 8 chunks across both HW DMA queues; the mask
        # load (128 tiny descriptors) goes on the gpsimd software queue so it
        # does not delay chunk 0 ---
        nc.gpsimd.dma_start(MM[:], m_d)
        for k, (t0, nt) in enumerate(CHUNKS):
            eng = nc.sync if k % 2 == 0 else nc.scalar
            src = d3a_d[k] if k < 2 else d3b_d[k - 2]
            eng.dma_start(D3[:, t0 * W:(t0 + nt) * W], src)

        # --- per chunk: upconvert trajectory cols to fp32 (ACT engine) ---
        for t0, nt in CHUNKS:
            src = bass.AP(D3.tensor, D3[:].offset + t0 * W,
                          [[T * W, PB], [1, NL], [W, nt]])
            dst = bass.AP(QL.tensor, QL[:].offset + t0,
                          [[NL * T, PB], [T, NL], [1, nt]])
            nc.scalar.activation(dst, src, AF.Copy)

        # --- per chunk: softmax denominators (cols 32..127 = exact row sum)
        # via a 2-level bf16 pairwise tree (DVE 2x mode) + reduce-24 ---
        for t0, nt in CHUNKS:
            def tseg(tile, per, off, width):
                return bass.AP(tile.tensor, tile[:].offset + off,
                               [[CT * per, PB], [per, nt], [1, width]])

            def dseg(off, width):
                return bass.AP(D3.tensor, D3[:].offset + t0 * W + off,
                               [[T * W, PB], [W, nt], [1, width]])

            nc.vector.tensor_tensor(tseg(DT1, 48, 0, 48), dseg(L, 48),
                                    dseg(L + 48, 48), op=ALU.add)
            nc.vector.tensor_tensor(tseg(DT2, 24, 0, 24), tseg(DT1, 48, 0, 24),
                                    tseg(DT1, 48, 24, 24), op=ALU.add)
            nc.vector.tensor_reduce(DG[:, t0:t0 + nt],
                                    tseg(DT2, 24, 0, 24),
                                    axis=mybir.AxisListType.X, op=ALU.add)
        nc.scalar.activation(LDG[:], DG[:], AF.Ln)

        # --- label-major DP: 33 iterations of (g-scan, u, f-scan), windowed
        # to the reachable band t in [l-1, T-L+l] (state 2l+1 is unreachable
        # before t=l and cannot complete the suffix after t=T-L+l; window
        # heads are genuine zeros computed by the previous l's scan, so no
        # re-zeroing is needed) ---
        def qwin(c, wl, wh):
            return QL[:, c * T + wl:c * T + wh + 1]

        prev = FD                        # h_l(t) = prev[:, t]
        for l in range(L + 1):
            wl = max(0, l - 1)
            wh = min(T - 1, T - L + l)
            h = prev[:, wl:wh + 1]
            nc.vector.tensor_tensor_scan(G[:, wl + 1:wh + 2], h, qwin(L, wl, wh),
                                         initial=0.0, op0=ALU.add, op1=ALU.mult)
            if l == L:
                break
            nc.vector.scalar_tensor_tensor(U[:, wl:wh + 1], h, MM[:, l:l + 1],
                                           G[:, wl:wh + 1],
                                           op0=ALU.mult, op1=ALU.add)
            cur = FF[l % 2]
            nc.vector.tensor_tensor_scan(cur[:, wl + 1:wh + 2], U[:, wl:wh + 1],
                                         qwin(l, wl, wh),
                                         initial=0.0, op0=ALU.add, op1=ALU.mult)
            prev = cur

        # --- epilogue: loss = SLD - ln(g_L(T-1) + f_{L-1}(T-1)) ---
        # ACT Ln saturates below ~2^-66; TOT spans ~2^-97..2^-40 on this data,
        # so evaluate ln(2^56 * TOT) and subtract 56*ln2 via the loss constant.
        nc.vector.reduce_sum(SLD[:], LDG[:], axis=mybir.AxisListType.X)
        fin = FF[(L - 1) % 2]
        nc.vector.tensor_tensor(TOT[:], G[:, T:T + 1], fin[:, T:T + 1],
                                op=ALU.add)
        nc.scalar.activation(LNT[:], TOT[:], AF.Ln, scale=float(2.0 ** 56))
        nc.vector.scalar_tensor_tensor(LOSS[:], SLD[:],
                                       float(56 * np.log(2.0)), LNT[:],
                                       op0=ALU.add, op1=ALU.subtract)
        for j in range(4):
            eng = nc.sync if j % 2 == 0 else nc.scalar
            eng.dma_start(loss_d[j * 32:(j + 1) * 32],
                          LOSS[j * 32:(j + 1) * 32, :])

    nc.compile()
    return nc


_prog_cache = {}


def _get_program():
    if "nc" not in _prog_cache:
        _prog_cache["nc"] = build_program()
    return _prog_cache["nc"]


def _core_in_maps(y_true, y_pred):
    y_true = np.asarray(y_true)
    y_pred = np.asarray(y_pred, dtype=np.float32)
    assert y_pred.shape == (B, T, C) and y_true.shape == (B, L)
    in_maps = []
    for cc in range(NCORES):
        sl = slice(cc * PB, (cc + 1) * PB)
        d3a, d3b, m = _pack_core_inputs(y_pred[sl], y_true[sl])
        in_maps.append({"d3a": d3a, "d3b": d3b, "m": m})
    return in_maps


def kernel(y_true, y_pred):
    nc = _get_program()
    res = run_bass_kernel_spmd(nc, _core_in_maps(y_true, y_pred),
                               list(range(NCORES)))
    out = np.concatenate([res.results[cc]["loss"] for cc in range(NCORES)],
                         axis=0)
    return out.astype(np.float32)


if __name__ == "__main__":
    rng = np.random.default_rng(0)
    yt = rng.integers(0, 95, (B, L)).astype(np.int32)
    yp = rng.uniform(0, 1, (B, T, C)).astype(np.float32)
    print(kernel(y_true=yt, y_pred=yp)[:4].ravel())
